# revision 20
# baseline (speedup 1.0000x reference)
"""Multi-head scaled-cosine attention (B=2, L=2048, E=2048, H=16, D=128) on 8 trn2 cores.

Sharding: core c = (b, g) with b = batch (2), g = head-group of 4 heads (4 groups).
Each core computes its 4 heads' attention for its batch plus the partial output
projection; the host sums the 4 per-group partials per batch.

v2: fp16 operands everywhere (same PE rate as bf16, 8x less rounding error),
DMA issue order tuned so the first projection matmul starts ~7us in (x block 0
and Wv/Wk stream before everything else), fp16 partial outputs (host sums in
f32), and a restructured attention phase: the previous q-chunk's output
projection is spread one chunk per k-tile inside the current q-chunk's loop
with a dedicated PSUM bank, scores rotate through 2 banks, and the softmax
denominator/normalizer drain is emitted ahead of the next chunk's work so PE
never waits on the bank.

Math identical to v1: RMS-norm cancels under L2 normalization; L2 reciprocal
and logit scale fold into the per-partition multiply before the PE transpose
producing Q^T/K^T; exp(bias - rowmax) is host-precomputed (fp16) and folded in
multiplicatively; scores build directly in [k, q] orientation; Q/K head dims
host-permuted (evens|odds) so RoPE uses contiguous vector ops.
"""
import sys
sys.path.insert(0, '/opt/trn_rl_repo')
import math
import numpy as np

import concourse.bacc as bacc
import concourse.mybir as mybir
import concourse.tile as tile
from concourse.bass_utils import run_bass_kernel_spmd

F32 = mybir.dt.float32
F32R = mybir.dt.float32r
FP16 = mybir.dt.float16
NP_FP16 = np.float16
ALU = mybir.AluOpType
AF = mybir.ActivationFunctionType

B, L, E, H, D = 2, 2048, 2048, 16, 128
G = 4                 # head groups
HPG = H // G          # heads per group = 4
GD = HPG * D          # 512, per-group projection width
P = 128               # partitions
NLT = L // P          # 16 l-tiles
NET = E // P          # 16 e-tiles (contraction)
NQC = L // 512        # 4 q-chunks
NKT = L // P          # 16 k-tiles
HD2 = GD // 2         # 256
LOGIT_SCALE_MAX = math.log(1.0 / 0.01)


def _build(apply_qs: bool, apply_ks: bool):
    nc = bacc.Bacc(None, target_bir_lowering=False)
    d = {}
    d['xqT'] = nc.dram_tensor("xqT", [E, L], FP16, kind="ExternalInput")
    d['xkvT'] = nc.dram_tensor("xkvT", [E, L], FP16, kind="ExternalInput")
    d['expBT'] = nc.dram_tensor("expBT", [L, L], FP16, kind="ExternalInput")
    d['wqT'] = nc.dram_tensor("wqT", [E, GD], FP16, kind="ExternalInput")
    d['wkT'] = nc.dram_tensor("wkT", [E, GD], FP16, kind="ExternalInput")
    d['wvT'] = nc.dram_tensor("wvT", [E, GD], FP16, kind="ExternalInput")
    d['woS'] = nc.dram_tensor("woS", [GD, E], FP16, kind="ExternalInput")
    d['c4q'] = nc.dram_tensor("c4q", [L, HD2], FP16, kind="ExternalInput")
    d['s4q'] = nc.dram_tensor("s4q", [L, HD2], FP16, kind="ExternalInput")
    d['c4k'] = nc.dram_tensor("c4k", [L, HD2], FP16, kind="ExternalInput")
    d['s4k'] = nc.dram_tensor("s4k", [L, HD2], FP16, kind="ExternalInput")
    d['ls'] = nc.dram_tensor("ls", [P, HPG], F32, kind="ExternalInput")
    if apply_qs:
        d['qscale'] = nc.dram_tensor("qscale", [P, GD], F32, kind="ExternalInput")
    if apply_ks:
        d['kscale'] = nc.dram_tensor("kscale", [P, GD], F32, kind="ExternalInput")
    out = nc.dram_tensor("out", [L, E], FP16, kind="ExternalOutput")

    with tile.TileContext(nc) as tc:
        with tc.tile_pool(name="persist", bufs=1) as persist:
            qT = [persist.tile([P, L], FP16, tag=f"qT{h}", name=f"qT{h}") for h in range(HPG)]
            kT = [persist.tile([P, L], FP16, tag=f"kT{h}", name=f"kT{h}") for h in range(HPG)]
            v_sb = persist.tile([P, NLT, GD], FP16, tag="v_sb")

            from contextlib import ExitStack
            proj_ctx = ExitStack()
            sbp = proj_ctx.enter_context(tc.tile_pool(name="proj_sb", bufs=4))
            nrm = proj_ctx.enter_context(tc.tile_pool(name="proj_nrm", bufs=6))
            psp = proj_ctx.enter_context(tc.tile_pool(name="proj_ps", bufs=4, space="PSUM"))
            pst = proj_ctx.enter_context(tc.tile_pool(name="proj_pst", bufs=4, space="PSUM"))

            # ---- DMA order: x block 0, then Wv/Wk, so the first matmul can
            # start after ~2.5MB of traffic instead of ~9MB.
            def load_xblk(x_dram, lt, name):
                blk = sbp.tile([P, NET, P], FP16, tag="xblk", name=name)
                nc.sync.dma_start(
                    blk[:],
                    x_dram[:, lt * P:(lt + 1) * P].rearrange("(g p) l -> p g l", p=P))
                return blk

            blk0 = load_xblk(d['xkvT'], 0, "xkvblk_0")

            # Wv/Wk in 4-e-tile chunks so the first projection matmuls can
            # start after ~1MB of DMA traffic.
            w_all = {}
            for wname in ('wvT', 'wkT'):
                w_all[wname] = persist.tile([P, NET, GD], FP16, tag=wname, name=f"w_{wname}")
            for ch in range(4):
                esl = slice(ch * 4 * P, (ch + 1) * 4 * P)
                for wname in ('wvT', 'wkT'):
                    nc.sync.dma_start(
                        w_all[wname][:, ch * 4:(ch + 1) * 4, :],
                        d[wname][esl, :].rearrange("(e p) n -> p e n", p=P))

            # small constants (cheap DMAs + on-chip setup, overlap with Wv/Wk)
            identh = persist.tile([P, P], FP16, tag="identh")
            identf = persist.tile([P, P], F32, tag="identf")
            nc.vector.memset(identf[:], 0.0)
            nc.gpsimd.affine_select(out=identf[:], in_=identf[:],
                                    compare_op=ALU.not_equal, fill=1.0, base=0,
                                    pattern=[[-1, P]], channel_multiplier=1)
            nc.vector.tensor_copy(identh[:], identf[:])
            ones_f = persist.tile([P, P], F32, tag="ones_f")
            nc.vector.memset(ones_f[:], 1.0)
            ones_r = persist.tile([P, P], F32R, tag="ones_r")
            nc.scalar.copy(ones_r[:], ones_f[:])
            ones_h = persist.tile([P, P], FP16, tag="ones_h")
            nc.vector.tensor_copy(ones_h[:], ones_f[:])
            ls_t = persist.tile([P, HPG], F32, tag="ls_t")
            nc.sync.dma_start(ls_t[:], d['ls'][:])

            qs_t = ks_t = None
            if apply_ks:
                ks_t = persist.tile([P, GD], F32, tag="ks_t")
                nc.sync.dma_start(ks_t[:], d['kscale'][:])

            def proj_psum(blk, w_sb, name):
                psum = psp.tile([P, GD], F32, tag="psum", name=name)
                for e in range(NET):
                    nc.tensor.matmul(psum[:], blk[:, e, :], w_sb[:, e, :],
                                     start=(e == 0), stop=(e == NET - 1))
                return psum

            def qk_norm(lt, psum, c_dram, s_dram, scale_tile, use_ls, dstT):
                q1 = nrm.tile([P, GD], FP16, tag="q1")
                nc.scalar.copy(q1[:], psum[:])
                if scale_tile is not None:
                    nc.vector.tensor_mul(q1[:], q1[:], scale_tile[:])
                ct = nrm.tile([P, HD2], FP16, tag="ct")
                st = nrm.tile([P, HD2], FP16, tag="st")
                nc.sync.dma_start(ct[:], c_dram[lt * P:(lt + 1) * P, :])
                nc.sync.dma_start(st[:], s_dram[lt * P:(lt + 1) * P, :])
                # per-head layout [evens(64) | odds(64)] (host-permuted weights)
                q1v = q1[:].rearrange("p (hh par dd) -> p hh par dd", hh=HPG, par=2)
                qe, qo = q1v[:, :, 0, :], q1v[:, :, 1, :]
                q2 = nrm.tile([P, GD], FP16, tag="q2")
                q2v = q2[:].rearrange("p (hh par dd) -> p hh par dd", hh=HPG, par=2)
                re, ro = q2v[:, :, 0, :], q2v[:, :, 1, :]
                ctv = ct[:].rearrange("p (hh dd) -> p hh dd", hh=HPG)
                stv = st[:].rearrange("p (hh dd) -> p hh dd", hh=HPG)
                tmp = nrm.tile([P, HD2], FP16, tag="tmp")
                tv = tmp[:].rearrange("p (hh dd) -> p hh dd", hh=HPG)
                # evens: qe*c - qo*s ; odds: qo*c + qe*s
                nc.vector.tensor_tensor(tv, qo, stv, ALU.mult)
                nc.vector.tensor_tensor(re, qe, ctv, ALU.mult)
                nc.vector.tensor_sub(re, re, tv)
                nc.vector.tensor_tensor(tv, qe, stv, ALU.mult)
                nc.vector.tensor_tensor(ro, qo, ctv, ALU.mult)
                nc.vector.tensor_add(ro, ro, tv)
                # L2 norm over each head's (now contiguous) D slice
                sqs = nrm.tile([P, GD], FP16, tag="sqs")
                acc = nrm.tile([P, HPG], F32, tag="acc")
                for h in range(HPG):
                    nc.scalar.activation(sqs[:, h * D:(h + 1) * D], q2[:, h * D:(h + 1) * D],
                                         AF.Square, accum_out=acc[:, h:h + 1])
                nrm_t = nrm.tile([P, HPG], F32, tag="nrm_t")
                nc.scalar.activation(nrm_t[:], acc[:], AF.Sqrt)
                nc.vector.tensor_scalar_max(nrm_t[:], nrm_t[:], 1e-12)
                rcp = nrm.tile([P, HPG], F32, tag="rcp")
                nc.vector.reciprocal(rcp[:], nrm_t[:])
                if use_ls:
                    nc.vector.tensor_mul(rcp[:], rcp[:], ls_t[:])
                q3 = nrm.tile([P, GD], FP16, tag="q3")
                for h in range(HPG):
                    nc.vector.tensor_scalar_mul(q3[:, h * D:(h + 1) * D],
                                                q2[:, h * D:(h + 1) * D], rcp[:, h:h + 1])
                for h in range(HPG):
                    pt = pst.tile([P, P], FP16, tag="pt", name=f"pt_{lt}_{h}")
                    nc.tensor.matmul(pt[:], q3[:, h * D:(h + 1) * D], identh[:],
                                     is_transpose=True)
                    nc.any.tensor_copy(dstT[h][:, lt * P:(lt + 1) * P], pt[:])

            # merged V+K phase: one xkvT block load feeds both projections
            for lt in range(NLT):
                blk = blk0 if lt == 0 else load_xblk(d['xkvT'], lt, f"xkvblk_{lt}")
                psum_v = proj_psum(blk, w_all['wvT'], f"psumv_{lt}")
                nc.scalar.copy(v_sb[:, lt, :], psum_v[:])
                psum_k = proj_psum(blk, w_all['wkT'], f"psumk_{lt}")
                qk_norm(lt, psum_k, d['c4k'], d['s4k'], ks_t, False, kT)
                if lt == 1:
                    # queue the Q/O weight loads behind the first x blocks
                    w_all['wqT'] = persist.tile([P, NET, GD], FP16, tag="wqT", name="w_wqT")
                    nc.sync.dma_start(
                        w_all['wqT'][:], d['wqT'][:].rearrange("(e p) n -> p e n", p=P))
                if lt == 3:
                    wo_sb = persist.tile([P, HPG, E], FP16, tag="wo_sb")
                    nc.sync.dma_start(
                        wo_sb[:], d['woS'][:].rearrange("(h p) e -> p h e", p=P))
                if lt == 2 and apply_qs:
                    qs_t = persist.tile([P, GD], F32, tag="qs_t")
                    nc.sync.dma_start(qs_t[:], d['qscale'][:])

            for lt in range(NLT):
                blk = load_xblk(d['xqT'], lt, f"xqblk_{lt}")
                psum = proj_psum(blk, w_all['wqT'], f"psumq_{lt}")
                qk_norm(lt, psum, d['c4q'], d['s4q'], qs_t, True, qT)
            proj_ctx.close()

            # ---- attention, one 512-wide q-chunk at a time ----
            # Heads are processed in two passes of 2 so the score matmuls get a
            # 4-deep PSUM rotation (decouples PE from the 558ns scalar exp):
            # banks = pv(2) + den(1) + outproj(1) + scores(4) = 8.
            att_ctx = ExitStack()
            asb = att_ctx.enter_context(tc.tile_pool(name="att_sb", bufs=3))
            atp = att_ctx.enter_context(tc.tile_pool(name="att_at", bufs=1))
            aop = att_ctx.enter_context(tc.tile_pool(name="att_o", bufs=3))
            ps_pv = att_ctx.enter_context(tc.tile_pool(name="ps_pv", bufs=1, space="PSUM"))
            ps_s = att_ctx.enter_context(tc.tile_pool(name="ps_s", bufs=2, space="PSUM"))
            ps_d = att_ctx.enter_context(tc.tile_pool(name="ps_d", bufs=1, space="PSUM"))
            ps_o = att_ctx.enter_context(tc.tile_pool(name="ps_o", bufs=1, space="PSUM"))

            _ocnt = [0]

            def outproj_chunk(qc, attn, lsub, ec, pool=None, tag="o_ps"):
                if pool is None:
                    o_ps = ps_o.tile([P, 512], F32, tag=tag, name=f"o{qc}_{lsub}_{ec}")
                else:
                    o_ps = pool.tile([P, 1024], F32, tag=tag,
                                     name=f"o{qc}_{lsub}_{ec}")[:, 0:512]
                for h in range(HPG):
                    nc.tensor.matmul(o_ps[:], attn[h][:, lsub * P:(lsub + 1) * P],
                                     wo_sb[:, h, ec * 512:(ec + 1) * 512],
                                     start=(h == 0), stop=(h == HPG - 1))
                o_sb = aop.tile([P, 512], FP16, tag="o_sb", name=f"ob{qc}_{lsub}_{ec}")
                nc.vector.tensor_copy(o_sb[:], o_ps[:])
                nc.sync.dma_start(
                    out[qc * 512 + lsub * P: qc * 512 + (lsub + 1) * P,
                        ec * 512:(ec + 1) * 512], o_sb[:])

            pending = None  # (qc, attn tiles) awaiting output projection
            for qc in range(NQC):
                qsl = slice(qc * 512, (qc + 1) * 512)
                # whole expB slab for this q-chunk in one DMA
                ebq = asb.tile([P, NKT, 512], FP16, tag="ebq", name=f"ebq{qc}", bufs=2)
                nc.sync.dma_start(
                    ebq[:], d['expBT'][:, qsl].rearrange("(kt p) q -> p kt q", p=P))
                den = ps_d.tile([P, 512], F32, tag="den", name=f"den{qc}")
                attn = [atp.tile([P, 512], FP16, tag=f"at{h}", name=f"at{qc}_{h}", bufs=2)
                        for h in range(HPG)]
                rcp32s = []

                for hp in range(2):           # head pair pass: heads 2*hp, 2*hp+1
                    h0 = 2 * hp
                    pv = [ps_pv.tile([P, 512], F32, tag=f"pv{i}", name=f"pv{qc}_{hp}_{i}")
                          for i in range(2)]

                    def stage1(kt):
                        p_t = asb.tile([P, 1024], FP16, tag="p_t",
                                       name=f"pt{qc}_{hp}_{kt}", bufs=4)
                        # both heads' scores into one 2-bank tile -> one exp
                        s_ps = ps_s.tile([P, 1024], F32, tag="s_ps",
                                         name=f"sp{qc}_{hp}_{kt}")
                        for i in range(2):
                            nc.tensor.matmul(s_ps[:, i * 512:(i + 1) * 512],
                                             kT[h0 + i][:, kt * P:(kt + 1) * P],
                                             qT[h0 + i][:, qsl], start=True, stop=True)
                        nc.scalar.activation(p_t[:], s_ps[:], AF.Exp)
                        ebb = ebq[:, kt, :].rearrange("p (o q) -> p o q", o=1) \
                                           .broadcast_to([P, 2, 512])
                        pv2 = p_t[:].rearrange("p (i q) -> p i q", i=2)
                        nc.vector.tensor_tensor(pv2, pv2, ebb, ALU.mult)
                        return p_t

                    def stage2(kt, p_t):
                        for i in range(2):
                            nc.tensor.matmul(pv[i][:],
                                             v_sb[:, kt, (h0 + i) * D:(h0 + i + 1) * D],
                                             p_t[:, i * 512:(i + 1) * 512],
                                             start=(kt == 0), stop=(kt == NKT - 1))

                    def den_acc(kt2, p_a, p_b):
                        # sum two k-tiles of p (fp16-safe: p <= 2*e^10),
                        # halving the PE den matmuls
                        p2 = asb.tile([P, 1024], FP16, tag="p2",
                                      name=f"p2_{qc}_{hp}_{kt2}", bufs=2)
                        nc.vector.tensor_add(p2[:], p_a[:], p_b[:])
                        for i in range(2):
                            off = 32 * (h0 + i)
                            nc.tensor.matmul(den[off:off + 32, :], ones_h[:, 0:32],
                                             p2[:, i * 512:(i + 1) * 512],
                                             start=(kt2 == 0), stop=(kt2 == NKT // 2 - 1),
                                             tile_position=(0, off))

                    hist = {}
                    for kt in range(NKT + 2):
                        if kt < NKT:
                            hist[kt] = stage1(kt)
                            if kt % 2 == 1:
                                den_acc(kt // 2, hist[kt - 1], hist[kt])
                        if kt >= 2:
                            stage2(kt - 2, hist[kt - 2])
                            del hist[kt - 2]
                        if pending is not None and kt % 2 == 0 and kt < NKT:
                            c = 8 * hp + kt // 2
                            outproj_chunk(pending[0], pending[1], c // 4, c % 4)
                            if c == 15:
                                pending = None

                    # pass drain: 1/den for this head pair, normalize straight
                    # out of the pv PSUM banks
                    lnd = asb.tile([64, 512], F32, tag=f"lnd{hp}", name=f"lnd{qc}_{hp}")
                    nc.scalar.activation(lnd[:], den[64 * hp:64 * hp + 64, :], AF.Ln)
                    rcp32 = asb.tile([64, 512], F32R, tag=f"rcp32{hp}",
                                     name=f"rcp32{qc}_{hp}")
                    nc.scalar.activation(rcp32[:], lnd[:], AF.Exp, scale=-1.0)
                    rcp32s.append(rcp32)
                    for i in range(2):
                        b_ps = ps_o.tile([P, 512], F32, tag="o_ps", name=f"b{qc}_{h0+i}")
                        off = 32 * i
                        nc.tensor.matmul(b_ps[:], ones_r[off:off + 1, :],
                                         rcp32[off:off + 1, :], start=True, stop=True)
                        rcpb = asb.tile([P, 512], FP16, tag="rcpb", name=f"rb{qc}_{h0+i}")
                        nc.vector.tensor_copy(rcpb[:], b_ps[:])
                        nc.vector.tensor_mul(attn[h0 + i][:], pv[i][:], rcpb[:])
                pending = (qc, attn)

            # last q-chunk's output projection, pipelined through the free
            # score banks
            qc, attn = pending
            for c in range(16):
                outproj_chunk(qc, attn, c // 4, c % 4, pool=ps_s, tag="s_ps")
            att_ctx.close()
    nc.compile()
    return nc


# head-dim permutation: within each head, evens first then odds
_PERM = np.empty(GD, np.int64)
for _i in range(GD):
    _h, _j = divmod(_i, D)
    _par, _dd = divmod(_j, D // 2)
    _PERM[_i] = _h * D + 2 * _dd + _par


def _prepare(inputs):
    f32 = np.float32
    inputs_q = np.asarray(inputs["inputs_q"], f32)
    inputs_kv = np.asarray(inputs["inputs_kv"], f32)
    bias = np.asarray(inputs["bias"], f32).reshape(L, L)
    q_sin = np.asarray(inputs["q_sinusoids"], f32)
    k_sin = np.asarray(inputs["k_sinusoids"], f32)
    Wq = np.asarray(inputs["Wq"], f32)
    Wk = np.asarray(inputs["Wk"], f32)
    Wv = np.asarray(inputs["Wv"], f32)
    Wo = np.asarray(inputs["Wo"], f32)
    qns = np.asarray(inputs["q_norm_scale"], f32)
    kns = np.asarray(inputs["k_norm_scale"], f32)
    ls = np.asarray(inputs["logit_scale"], f32)

    apply_qs = not np.all(qns == 1.0)
    apply_ks = not np.all(kns == 1.0)

    bm = bias.max(axis=1, keepdims=True)
    expBT = np.ascontiguousarray(np.exp((bias - bm).T).astype(NP_FP16))
    ls_e = np.exp(np.minimum(ls, LOGIT_SCALE_MAX)).astype(f32)

    per_b = []
    for b in range(B):
        per_b.append(dict(
            xqT=np.ascontiguousarray(inputs_q[b].T.astype(NP_FP16)),
            xkvT=np.ascontiguousarray(inputs_kv[b].T.astype(NP_FP16)),
            c4q=np.ascontiguousarray(np.tile(q_sin[b][:, 0::2], (1, HPG)).astype(NP_FP16)),
            s4q=np.ascontiguousarray(np.tile(q_sin[b][:, 1::2], (1, HPG)).astype(NP_FP16)),
            c4k=np.ascontiguousarray(np.tile(k_sin[b][:, 0::2], (1, HPG)).astype(NP_FP16)),
            s4k=np.ascontiguousarray(np.tile(k_sin[b][:, 1::2], (1, HPG)).astype(NP_FP16)),
        ))
    per_g = []
    for g in range(G):
        rows = slice(g * GD, (g + 1) * GD)
        per_g.append(dict(
            wqT=np.ascontiguousarray(Wq[rows, :][_PERM, :].T.astype(NP_FP16)),
            wkT=np.ascontiguousarray(Wk[rows, :][_PERM, :].T.astype(NP_FP16)),
            wvT=np.ascontiguousarray(Wv[rows, :].T.astype(NP_FP16)),
            woS=np.ascontiguousarray(Wo[:, rows].T.astype(NP_FP16)),
            ls=np.broadcast_to(ls_e[g * HPG:(g + 1) * HPG][None, :], (P, HPG)).copy(),
        ))

    qs_bc = (np.broadcast_to(np.tile(qns, HPG)[_PERM][None, :], (P, GD)).copy()
             if apply_qs else None)
    ks_bc = (np.broadcast_to(np.tile(kns, HPG)[_PERM][None, :], (P, GD)).copy()
             if apply_ks else None)

    in_maps = []
    for c in range(8):
        b, g = divmod(c, G)
        m = dict(expBT=expBT)
        m.update(per_b[b])
        m.update(per_g[g])
        if apply_qs:
            m['qscale'] = qs_bc
        if apply_ks:
            m['kscale'] = ks_bc
        in_maps.append(m)
    return in_maps, apply_qs, apply_ks


_CACHE = {}


def _get_nc(apply_qs, apply_ks):
    key = (apply_qs, apply_ks)
    if key not in _CACHE:
        _CACHE[key] = _build(apply_qs, apply_ks)
    return _CACHE[key]


def kernel(**inputs) -> np.ndarray:
    in_maps, apply_qs, apply_ks = _prepare(inputs)
    nc = _get_nc(apply_qs, apply_ks)
    res = run_bass_kernel_spmd(nc, in_maps, core_ids=list(range(8)))
    out = np.zeros((B, L, E), np.float32)
    for c in range(8):
        b = c // G
        out[b] += res.results[c]["out"].astype(np.float32)
    return out


# revision 24
# speedup vs baseline: 1.2074x; 1.2074x over previous
"""Multi-head scaled-cosine attention (B=2, L=2048, E=2048, H=16, D=128) on 8 trn2 cores.

Sharding: core c = (b, g) with b = batch (2), g = head-group of 4 heads (4 groups).
Each core computes its 4 heads' attention for its batch plus the partial output
projection; the host sums the 4 per-group partials per batch.

v2: fp16 operands everywhere (same PE rate as bf16, 8x less rounding error),
DMA issue order tuned so the first projection matmul starts ~7us in (x block 0
and Wv/Wk stream before everything else), fp16 partial outputs (host sums in
f32), and a restructured attention phase: the previous q-chunk's output
projection is spread one chunk per k-tile inside the current q-chunk's loop
with a dedicated PSUM bank, scores rotate through 2 banks, and the softmax
denominator/normalizer drain is emitted ahead of the next chunk's work so PE
never waits on the bank.

Math identical to v1: RMS-norm cancels under L2 normalization; L2 reciprocal
and logit scale fold into the per-partition multiply before the PE transpose
producing Q^T/K^T; exp(bias - rowmax) is host-precomputed (fp16) and folded in
multiplicatively; scores build directly in [k, q] orientation; Q/K head dims
host-permuted (evens|odds) so RoPE uses contiguous vector ops.
"""
import sys
sys.path.insert(0, '/opt/trn_rl_repo')
import math
import numpy as np

import concourse.bacc as bacc
import concourse.mybir as mybir
import concourse.tile as tile
from concourse.bass_utils import run_bass_kernel_spmd

F32 = mybir.dt.float32
F32R = mybir.dt.float32r
FP16 = mybir.dt.float16
NP_FP16 = np.float16
ALU = mybir.AluOpType
AF = mybir.ActivationFunctionType

B, L, E, H, D = 2, 2048, 2048, 16, 128
G = 4                 # head groups
HPG = H // G          # heads per group = 4
GD = HPG * D          # 512, per-group projection width
P = 128               # partitions
NLT = L // P          # 16 l-tiles
NET = E // P          # 16 e-tiles (contraction)
NQC = L // 512        # 4 q-chunks
NKT = L // P          # 16 k-tiles
HD2 = GD // 2         # 256
LOGIT_SCALE_MAX = math.log(1.0 / 0.01)


def _build(apply_qs: bool, apply_ks: bool):
    nc = bacc.Bacc(None, target_bir_lowering=False)
    d = {}
    d['xqT'] = nc.dram_tensor("xqT", [E, L], FP16, kind="ExternalInput")
    d['xkvT'] = nc.dram_tensor("xkvT", [E, L], FP16, kind="ExternalInput")
    d['expBT'] = nc.dram_tensor("expBT", [L, L], FP16, kind="ExternalInput")
    d['wqT'] = nc.dram_tensor("wqT", [E, GD], FP16, kind="ExternalInput")
    d['wkT'] = nc.dram_tensor("wkT", [E, GD], FP16, kind="ExternalInput")
    d['wvT'] = nc.dram_tensor("wvT", [E, GD], FP16, kind="ExternalInput")
    d['woS'] = nc.dram_tensor("woS", [GD, E], FP16, kind="ExternalInput")
    d['c4q'] = nc.dram_tensor("c4q", [L, HD2], FP16, kind="ExternalInput")
    d['s4q'] = nc.dram_tensor("s4q", [L, HD2], FP16, kind="ExternalInput")
    d['c4k'] = nc.dram_tensor("c4k", [L, HD2], FP16, kind="ExternalInput")
    d['s4k'] = nc.dram_tensor("s4k", [L, HD2], FP16, kind="ExternalInput")
    d['ls'] = nc.dram_tensor("ls", [P, HPG], F32, kind="ExternalInput")
    if apply_qs:
        d['qscale'] = nc.dram_tensor("qscale", [P, GD], F32, kind="ExternalInput")
    if apply_ks:
        d['kscale'] = nc.dram_tensor("kscale", [P, GD], F32, kind="ExternalInput")
    out = nc.dram_tensor("out", [L, E], FP16, kind="ExternalOutput")

    with tile.TileContext(nc) as tc:
        with tc.tile_pool(name="persist", bufs=1) as persist:
            qT = [persist.tile([P, L], FP16, tag=f"qT{h}", name=f"qT{h}") for h in range(HPG)]
            kT = [persist.tile([P, L], FP16, tag=f"kT{h}", name=f"kT{h}") for h in range(HPG)]
            v_sb = persist.tile([P, NLT, GD], FP16, tag="v_sb")

            from contextlib import ExitStack
            proj_ctx = ExitStack()
            sbp = proj_ctx.enter_context(tc.tile_pool(name="proj_sb", bufs=4))
            nrm = proj_ctx.enter_context(tc.tile_pool(name="proj_nrm", bufs=6))
            psp = proj_ctx.enter_context(tc.tile_pool(name="proj_ps", bufs=4, space="PSUM"))
            pst = proj_ctx.enter_context(tc.tile_pool(name="proj_pst", bufs=4, space="PSUM"))

            # ---- DMA order: x block 0, then Wv/Wk, so the first matmul can
            # start after ~2.5MB of traffic instead of ~9MB.
            def load_xblk(x_dram, lt, name):
                blk = sbp.tile([P, NET, P], FP16, tag="xblk", name=name)
                nc.sync.dma_start(
                    blk[:],
                    x_dram[:, lt * P:(lt + 1) * P].rearrange("(g p) l -> p g l", p=P))
                return blk

            blk0 = load_xblk(d['xkvT'], 0, "xkvblk_0")

            # Wv/Wk in 4-e-tile chunks so the first projection matmuls can
            # start after ~1MB of DMA traffic.
            w_all = {}
            for wname in ('wvT', 'wkT'):
                w_all[wname] = persist.tile([P, NET, GD], FP16, tag=wname, name=f"w_{wname}")
            for ch in range(4):
                esl = slice(ch * 4 * P, (ch + 1) * 4 * P)
                for wname in ('wvT', 'wkT'):
                    nc.sync.dma_start(
                        w_all[wname][:, ch * 4:(ch + 1) * 4, :],
                        d[wname][esl, :].rearrange("(e p) n -> p e n", p=P))

            # small constants (cheap DMAs + on-chip setup, overlap with Wv/Wk)
            identh = persist.tile([P, P], FP16, tag="identh")
            identf = persist.tile([P, P], F32, tag="identf")
            nc.vector.memset(identf[:], 0.0)
            nc.gpsimd.affine_select(out=identf[:], in_=identf[:],
                                    compare_op=ALU.not_equal, fill=1.0, base=0,
                                    pattern=[[-1, P]], channel_multiplier=1)
            nc.vector.tensor_copy(identh[:], identf[:])
            ones_f = persist.tile([P, P], F32, tag="ones_f")
            nc.vector.memset(ones_f[:], 1.0)
            ones_r = persist.tile([P, P], F32R, tag="ones_r")
            nc.scalar.copy(ones_r[:], ones_f[:])
            ones_h = persist.tile([P, P], FP16, tag="ones_h")
            nc.vector.tensor_copy(ones_h[:], ones_f[:])
            ls_t = persist.tile([P, HPG], F32, tag="ls_t")
            nc.sync.dma_start(ls_t[:], d['ls'][:])

            qs_t = ks_t = None
            if apply_ks:
                ks_t = persist.tile([P, GD], F32, tag="ks_t")
                nc.sync.dma_start(ks_t[:], d['kscale'][:])

            def proj_psum(blk, w_sb, name):
                psum = psp.tile([P, GD], F32, tag="psum", name=name)
                for e in range(NET):
                    nc.tensor.matmul(psum[:], blk[:, e, :], w_sb[:, e, :],
                                     start=(e == 0), stop=(e == NET - 1))
                return psum

            def qk_norm(lt, psum, c_dram, s_dram, scale_tile, use_ls, dstT):
                q1 = nrm.tile([P, GD], FP16, tag="q1")
                nc.scalar.copy(q1[:], psum[:])
                if scale_tile is not None:
                    nc.vector.tensor_mul(q1[:], q1[:], scale_tile[:])
                ct = nrm.tile([P, HD2], FP16, tag="ct")
                st = nrm.tile([P, HD2], FP16, tag="st")
                nc.sync.dma_start(ct[:], c_dram[lt * P:(lt + 1) * P, :])
                nc.sync.dma_start(st[:], s_dram[lt * P:(lt + 1) * P, :])
                # per-head layout [evens(64) | odds(64)] (host-permuted weights)
                q1v = q1[:].rearrange("p (hh par dd) -> p hh par dd", hh=HPG, par=2)
                qe, qo = q1v[:, :, 0, :], q1v[:, :, 1, :]
                q2 = nrm.tile([P, GD], FP16, tag="q2")
                q2v = q2[:].rearrange("p (hh par dd) -> p hh par dd", hh=HPG, par=2)
                re, ro = q2v[:, :, 0, :], q2v[:, :, 1, :]
                ctv = ct[:].rearrange("p (hh dd) -> p hh dd", hh=HPG)
                stv = st[:].rearrange("p (hh dd) -> p hh dd", hh=HPG)
                tmp = nrm.tile([P, HD2], FP16, tag="tmp")
                tv = tmp[:].rearrange("p (hh dd) -> p hh dd", hh=HPG)
                # evens: qe*c - qo*s ; odds: qo*c + qe*s
                nc.vector.tensor_tensor(tv, qo, stv, ALU.mult)
                nc.vector.tensor_tensor(re, qe, ctv, ALU.mult)
                nc.vector.tensor_sub(re, re, tv)
                nc.vector.tensor_tensor(tv, qe, stv, ALU.mult)
                nc.vector.tensor_tensor(ro, qo, ctv, ALU.mult)
                nc.vector.tensor_add(ro, ro, tv)
                # L2 norm over each head's (now contiguous) D slice
                sqs = nrm.tile([P, GD], FP16, tag="sqs")
                acc = nrm.tile([P, HPG], F32, tag="acc")
                for h in range(HPG):
                    nc.scalar.activation(sqs[:, h * D:(h + 1) * D], q2[:, h * D:(h + 1) * D],
                                         AF.Square, accum_out=acc[:, h:h + 1])
                nrm_t = nrm.tile([P, HPG], F32, tag="nrm_t")
                nc.scalar.activation(nrm_t[:], acc[:], AF.Sqrt)
                nc.vector.tensor_scalar_max(nrm_t[:], nrm_t[:], 1e-12)
                rcp = nrm.tile([P, HPG], F32, tag="rcp")
                nc.vector.reciprocal(rcp[:], nrm_t[:])
                if use_ls:
                    nc.vector.tensor_mul(rcp[:], rcp[:], ls_t[:])
                q3 = nrm.tile([P, GD], FP16, tag="q3")
                for h in range(HPG):
                    nc.vector.tensor_scalar_mul(q3[:, h * D:(h + 1) * D],
                                                q2[:, h * D:(h + 1) * D], rcp[:, h:h + 1])
                for h in range(HPG):
                    pt = pst.tile([P, P], FP16, tag="pt", name=f"pt_{lt}_{h}")
                    nc.tensor.matmul(pt[:], q3[:, h * D:(h + 1) * D], identh[:],
                                     is_transpose=True)
                    nc.any.tensor_copy(dstT[h][:, lt * P:(lt + 1) * P], pt[:])

            # merged V+K phase: one xkvT block load feeds both projections
            for lt in range(NLT):
                blk = blk0 if lt == 0 else load_xblk(d['xkvT'], lt, f"xkvblk_{lt}")
                psum_v = proj_psum(blk, w_all['wvT'], f"psumv_{lt}")
                nc.scalar.copy(v_sb[:, lt, :], psum_v[:])
                psum_k = proj_psum(blk, w_all['wkT'], f"psumk_{lt}")
                qk_norm(lt, psum_k, d['c4k'], d['s4k'], ks_t, False, kT)
                if lt == 1:
                    # queue the Q/O weight loads behind the first x blocks
                    w_all['wqT'] = persist.tile([P, NET, GD], FP16, tag="wqT", name="w_wqT")
                    nc.sync.dma_start(
                        w_all['wqT'][:], d['wqT'][:].rearrange("(e p) n -> p e n", p=P))
                if lt == 3:
                    wo_sb = persist.tile([P, HPG, E], FP16, tag="wo_sb")
                    nc.sync.dma_start(
                        wo_sb[:], d['woS'][:].rearrange("(h p) e -> p h e", p=P))
                if lt == 2 and apply_qs:
                    qs_t = persist.tile([P, GD], F32, tag="qs_t")
                    nc.sync.dma_start(qs_t[:], d['qscale'][:])

            for lt in range(NLT):
                blk = load_xblk(d['xqT'], lt, f"xqblk_{lt}")
                psum = proj_psum(blk, w_all['wqT'], f"psumq_{lt}")
                qk_norm(lt, psum, d['c4q'], d['s4q'], qs_t, True, qT)
            proj_ctx.close()

            # ---- attention, one 512-wide q-chunk at a time ----
            # Heads are processed in two passes of 2 so the score matmuls get a
            # 4-deep PSUM rotation (decouples PE from the 558ns scalar exp):
            # banks = pv(2) + den(1) + outproj(1) + scores(4) = 8.
            att_ctx = ExitStack()
            asb = att_ctx.enter_context(tc.tile_pool(name="att_sb", bufs=3))
            atp = att_ctx.enter_context(tc.tile_pool(name="att_at", bufs=1))
            aop = att_ctx.enter_context(tc.tile_pool(name="att_o", bufs=3))
            ps_pv = att_ctx.enter_context(tc.tile_pool(name="ps_pv", bufs=1, space="PSUM"))
            ps_s = att_ctx.enter_context(tc.tile_pool(name="ps_s", bufs=2, space="PSUM"))
            ps_d = att_ctx.enter_context(tc.tile_pool(name="ps_d", bufs=1, space="PSUM"))
            ps_o = att_ctx.enter_context(tc.tile_pool(name="ps_o", bufs=1, space="PSUM"))

            _ocnt = [0]

            def outproj_chunk(qc, attn, lsub, ec, pool=None, tag="o_ps"):
                if pool is None:
                    o_ps = ps_o.tile([P, 512], F32, tag=tag, name=f"o{qc}_{lsub}_{ec}")
                else:
                    o_ps = pool.tile([P, 1024], F32, tag=tag,
                                     name=f"o{qc}_{lsub}_{ec}")[:, 0:512]
                for h in range(HPG):
                    nc.tensor.matmul(o_ps[:], attn[h][:, lsub * P:(lsub + 1) * P],
                                     wo_sb[:, h, ec * 512:(ec + 1) * 512],
                                     start=(h == 0), stop=(h == HPG - 1))
                o_sb = aop.tile([P, 512], FP16, tag="o_sb", name=f"ob{qc}_{lsub}_{ec}")
                nc.vector.tensor_copy(o_sb[:], o_ps[:])
                nc.sync.dma_start(
                    out[qc * 512 + lsub * P: qc * 512 + (lsub + 1) * P,
                        ec * 512:(ec + 1) * 512], o_sb[:])

            pending = None  # (qc, attn tiles) awaiting output projection
            deferred = []   # per-head normalize work whose deps need time to settle
            for qc in range(NQC):
                qsl = slice(qc * 512, (qc + 1) * 512)
                # whole expB slab for this q-chunk in one DMA
                ebq = asb.tile([P, NKT, 512], FP16, tag="ebq", name=f"ebq{qc}", bufs=2)
                nc.sync.dma_start(
                    ebq[:], d['expBT'][:, qsl].rearrange("(kt p) q -> p kt q", p=P))
                den = ps_d.tile([P, 512], F32, tag="den", name=f"den{qc}")
                attn = [atp.tile([P, 512], FP16, tag=f"at{h}", name=f"at{qc}_{h}", bufs=2)
                        for h in range(HPG)]

                for hp in range(2):           # head pair pass: heads 2*hp, 2*hp+1
                    h0 = 2 * hp
                    pv = [ps_pv.tile([P, 512], F32, tag=f"pv{i}", name=f"pv{qc}_{hp}_{i}")
                          for i in range(2)]

                    def stage1(kt):
                        p_t = asb.tile([P, 1024], FP16, tag="p_t",
                                       name=f"pt{qc}_{hp}_{kt}", bufs=4)
                        # both heads' scores into one 2-bank tile -> one exp
                        s_ps = ps_s.tile([P, 1024], F32, tag="s_ps",
                                         name=f"sp{qc}_{hp}_{kt}")
                        for i in range(2):
                            nc.tensor.matmul(s_ps[:, i * 512:(i + 1) * 512],
                                             kT[h0 + i][:, kt * P:(kt + 1) * P],
                                             qT[h0 + i][:, qsl], start=True, stop=True)
                        nc.scalar.activation(p_t[:], s_ps[:], AF.Exp)
                        ebb = ebq[:, kt, :].rearrange("p (o q) -> p o q", o=1) \
                                           .broadcast_to([P, 2, 512])
                        pv2 = p_t[:].rearrange("p (i q) -> p i q", i=2)
                        nc.vector.tensor_tensor(pv2, pv2, ebb, ALU.mult)
                        return p_t

                    def stage2(kt, p_t):
                        for i in range(2):
                            nc.tensor.matmul(pv[i][:],
                                             v_sb[:, kt, (h0 + i) * D:(h0 + i + 1) * D],
                                             p_t[:, i * 512:(i + 1) * 512],
                                             start=(kt == 0), stop=(kt == NKT - 1))

                    def den_acc(kt2, p_a, p_b):
                        # sum two k-tiles of p (fp16-safe: p <= 2*e^10),
                        # halving the PE den matmuls
                        p2 = asb.tile([P, 1024], FP16, tag="p2",
                                      name=f"p2_{qc}_{hp}_{kt2}", bufs=2)
                        nc.vector.tensor_add(p2[:], p_a[:], p_b[:])
                        for i in range(2):
                            off = 32 * (h0 + i)
                            nc.tensor.matmul(den[off:off + 32, :], ones_h[:, 0:32],
                                             p2[:, i * 512:(i + 1) * 512],
                                             start=(kt2 == 0), stop=(kt2 == NKT // 2 - 1),
                                             tile_position=(0, off))

                    hist = {}
                    for kt in range(NKT + 2):
                        if kt < NKT:
                            hist[kt] = stage1(kt)
                            if kt % 2 == 1:
                                den_acc(kt // 2, hist[kt - 1], hist[kt])
                        if kt >= 2:
                            stage2(kt - 2, hist[kt - 2])
                            del hist[kt - 2]
                        if kt == 1:
                            while deferred:
                                deferred.pop(0)()
                        if pending is not None and kt % 2 == 0 and 2 <= kt <= 16:
                            c = 8 * hp + (kt - 2) // 2
                            outproj_chunk(pending[0], pending[1], c // 4, c % 4)
                            if c == 15:
                                pending = None

                    # pass drain: release pv banks via fast copies, compute
                    # 1/den, and DEFER the normalize (which contains a PE
                    # matmul behind the scalar chain) into the next pass.
                    pvc = []
                    for i in range(2):
                        c = asb.tile([P, 512], FP16, tag=f"pvc{i}",
                                     name=f"pvc{qc}_{hp}_{i}", bufs=2)
                        nc.vector.tensor_copy(c[:], pv[i][:])
                        pvc.append(c)
                    lnd = asb.tile([64, 512], F32, tag=f"lnd{hp}", name=f"lnd{qc}_{hp}")
                    nc.scalar.activation(lnd[:], den[64 * hp:64 * hp + 64, :], AF.Ln)
                    rcp32 = asb.tile([64, 512], F32R, tag=f"rcp32{hp}",
                                     name=f"rcp32{qc}_{hp}")
                    nc.scalar.activation(rcp32[:], lnd[:], AF.Exp, scale=-1.0)

                    def make_norm(qc_, h0_, rcp32_, pvc_, attn_):
                        def norm(i):
                            def run():
                                b_ps = ps_o.tile([P, 512], F32, tag="o_ps",
                                                 name=f"b{qc_}_{h0_ + i}")
                                off = 32 * i
                                nc.tensor.matmul(b_ps[:], ones_r[off:off + 1, :],
                                                 rcp32_[off:off + 1, :],
                                                 start=True, stop=True)
                                rcpb = asb.tile([P, 512], FP16, tag="rcpb",
                                                name=f"rb{qc_}_{h0_ + i}")
                                nc.vector.tensor_copy(rcpb[:], b_ps[:])
                                nc.vector.tensor_mul(attn_[h0_ + i][:], pvc_[i][:],
                                                     rcpb[:])
                            return run
                        return [norm(0), norm(1)]
                    deferred.extend(make_norm(qc, h0, rcp32, pvc, attn))
                pending = (qc, attn)

            # flush remaining deferred normalizes, then the last q-chunk's
            # output projection pipelined through the free score banks
            for fn in deferred:
                fn()
            deferred = []
            qc, attn = pending
            for c in range(16):
                outproj_chunk(qc, attn, c // 4, c % 4, pool=ps_s, tag="s_ps")
            att_ctx.close()
    nc.compile()
    return nc


# head-dim permutation: within each head, evens first then odds
_PERM = np.empty(GD, np.int64)
for _i in range(GD):
    _h, _j = divmod(_i, D)
    _par, _dd = divmod(_j, D // 2)
    _PERM[_i] = _h * D + 2 * _dd + _par


def _prepare(inputs):
    f32 = np.float32
    inputs_q = np.asarray(inputs["inputs_q"], f32)
    inputs_kv = np.asarray(inputs["inputs_kv"], f32)
    bias = np.asarray(inputs["bias"], f32).reshape(L, L)
    q_sin = np.asarray(inputs["q_sinusoids"], f32)
    k_sin = np.asarray(inputs["k_sinusoids"], f32)
    Wq = np.asarray(inputs["Wq"], f32)
    Wk = np.asarray(inputs["Wk"], f32)
    Wv = np.asarray(inputs["Wv"], f32)
    Wo = np.asarray(inputs["Wo"], f32)
    qns = np.asarray(inputs["q_norm_scale"], f32)
    kns = np.asarray(inputs["k_norm_scale"], f32)
    ls = np.asarray(inputs["logit_scale"], f32)

    apply_qs = not np.all(qns == 1.0)
    apply_ks = not np.all(kns == 1.0)

    bm = bias.max(axis=1, keepdims=True)
    expBT = np.ascontiguousarray(np.exp((bias - bm).T).astype(NP_FP16))
    ls_e = np.exp(np.minimum(ls, LOGIT_SCALE_MAX)).astype(f32)

    per_b = []
    for b in range(B):
        per_b.append(dict(
            xqT=np.ascontiguousarray(inputs_q[b].T.astype(NP_FP16)),
            xkvT=np.ascontiguousarray(inputs_kv[b].T.astype(NP_FP16)),
            c4q=np.ascontiguousarray(np.tile(q_sin[b][:, 0::2], (1, HPG)).astype(NP_FP16)),
            s4q=np.ascontiguousarray(np.tile(q_sin[b][:, 1::2], (1, HPG)).astype(NP_FP16)),
            c4k=np.ascontiguousarray(np.tile(k_sin[b][:, 0::2], (1, HPG)).astype(NP_FP16)),
            s4k=np.ascontiguousarray(np.tile(k_sin[b][:, 1::2], (1, HPG)).astype(NP_FP16)),
        ))
    per_g = []
    for g in range(G):
        rows = slice(g * GD, (g + 1) * GD)
        per_g.append(dict(
            wqT=np.ascontiguousarray(Wq[rows, :][_PERM, :].T.astype(NP_FP16)),
            wkT=np.ascontiguousarray(Wk[rows, :][_PERM, :].T.astype(NP_FP16)),
            wvT=np.ascontiguousarray(Wv[rows, :].T.astype(NP_FP16)),
            woS=np.ascontiguousarray(Wo[:, rows].T.astype(NP_FP16)),
            ls=np.broadcast_to(ls_e[g * HPG:(g + 1) * HPG][None, :], (P, HPG)).copy(),
        ))

    qs_bc = (np.broadcast_to(np.tile(qns, HPG)[_PERM][None, :], (P, GD)).copy()
             if apply_qs else None)
    ks_bc = (np.broadcast_to(np.tile(kns, HPG)[_PERM][None, :], (P, GD)).copy()
             if apply_ks else None)

    in_maps = []
    for c in range(8):
        b, g = divmod(c, G)
        m = dict(expBT=expBT)
        m.update(per_b[b])
        m.update(per_g[g])
        if apply_qs:
            m['qscale'] = qs_bc
        if apply_ks:
            m['kscale'] = ks_bc
        in_maps.append(m)
    return in_maps, apply_qs, apply_ks


_CACHE = {}


def _get_nc(apply_qs, apply_ks):
    key = (apply_qs, apply_ks)
    if key not in _CACHE:
        _CACHE[key] = _build(apply_qs, apply_ks)
    return _CACHE[key]


def kernel(**inputs) -> np.ndarray:
    in_maps, apply_qs, apply_ks = _prepare(inputs)
    nc = _get_nc(apply_qs, apply_ks)
    res = run_bass_kernel_spmd(nc, in_maps, core_ids=list(range(8)))
    out = np.zeros((B, L, E), np.float32)
    for c in range(8):
        b = c // G
        out[b] += res.results[c]["out"].astype(np.float32)
    return out
